# revision 1
# baseline (speedup 1.0000x reference)
"""Bidirectional WKV (Vision-RWKV style) Trainium2 kernel.

Full-input contract: kernel(**inputs) takes the unsharded inputs as numpy
arrays and returns the full [B, T, C] output. Internally shards data-parallel
over B across 8 NeuronCores; each core handles one batch element.

Math: the reference's max-stabilized scans cancel exactly; we compute
  sL_t = e^{-w} sL_{t-1} + e^{k_t} v_t   (inclusive fwd scan;  Sl_t = sL_{t-1})
  sR_t = e^{-w} sR_{t+1} + e^{k_t} v_t   (inclusive bwd scan;  Sr_t = sR_{t+1})
  y_t  = (Sl_t + Sr_t + e^u e^{k_t} v_t) / (SzL_t + SzR_t + e^u e^{k_t})
  out  = (sigmoid(r) * y) @ Wo^T
Exponents stay within ~e^{+-12}, so fp32 is safe without per-step rescaling.
"""

import numpy as np
from contextlib import ExitStack

import concourse.bass as bass
import concourse.tile as tile
from concourse import bacc, mybir
from concourse.bass_utils import run_bass_kernel_spmd

B, T, C = 8, 2048, 768
N_CORES = 8
P = 128
NC_TILES = C // P          # 6 channel tiles
NCHUNK = 512               # matmul moving-dim chunk
NT = T // NCHUNK           # 4 time chunks
F32 = mybir.dt.float32
F32R = mybir.dt.float32r
AF = mybir.ActivationFunctionType
OP = mybir.AluOpType

# build-time knobs
MM_BITCAST = F32R          # matmul input dtype (f32r: full PE rate, ~f32 precision)
XT_RESIDENT = True         # keep all of xT in SBUF


def _slc(i, n):
    return slice(i * n, (i + 1) * n)


def build_kernel():
    """Build + compile the Bass module (single program, run SPMD on 8 cores)."""
    nc = bacc.Bacc("TRN2", target_bir_lowering=False, debug=False,
                   num_devices=N_CORES)
    xT = nc.dram_tensor("xT", [C, T], MM_BITCAST, kind="ExternalInput").ap()
    wkT = nc.dram_tensor("WkT", [C, C], MM_BITCAST, kind="ExternalInput").ap()
    wvT = nc.dram_tensor("WvT", [C, C], MM_BITCAST, kind="ExternalInput").ap()
    wrT = nc.dram_tensor("WrT", [C, C], MM_BITCAST, kind="ExternalInput").ap()
    woT = nc.dram_tensor("WoT", [C, C], MM_BITCAST, kind="ExternalInput").ap()
    ew = nc.dram_tensor("ew", [C, 1], F32, kind="ExternalInput").ap()   # exp(-decay/T)
    eu = nc.dram_tensor("eu", [C, 1], F32, kind="ExternalInput").ap()   # exp(first/T)
    outT = nc.dram_tensor("outT", [C, T], F32, kind="ExternalOutput").ap()

    with tile.TileContext(nc) as tc:
        with ExitStack() as ctx:
            const = ctx.enter_context(tc.tile_pool(name="const", bufs=1))
            xpool = ctx.enter_context(tc.tile_pool(name="xpool", bufs=1))
            zpool = ctx.enter_context(tc.tile_pool(name="zpool", bufs=1))
            work = ctx.enter_context(tc.tile_pool(name="work", bufs=2))
            scanp = ctx.enter_context(tc.tile_pool(name="scanp", bufs=1))
            wpool = ctx.enter_context(tc.tile_pool(name="wpool", bufs=8))
            opool = ctx.enter_context(tc.tile_pool(name="opool", bufs=2))
            psum = ctx.enter_context(tc.tile_pool(name="psum", bufs=8, space="PSUM"))

            # per-channel constants as [P, NC_TILES]
            ew_sb = const.tile([P, NC_TILES], F32, tag="ew")
            eu_sb = const.tile([P, NC_TILES], F32, tag="eu")
            nc.sync.dma_start(ew_sb[:], ew.rearrange("(j p) o -> p (j o)", p=P))
            nc.sync.dma_start(eu_sb[:], eu.rearrange("(j p) o -> p (j o)", p=P))

            # resident x^T [c, t] tiles
            xsb = []
            for ci in range(NC_TILES):
                xt = xpool.tile([P, T], MM_BITCAST, tag=f"x{ci}")
                nc.sync.dma_start(xt[:], xT[_slc(ci, P), :])
                xsb.append(xt)

            zts = []
            for j in range(NC_TILES):
                # ---- projections for channel block j ----
                ek = work.tile([P, T], F32, tag="ek")
                ekv = work.tile([P, T], F32, tag="ekv")
                sr = work.tile([P, T], F32, tag="sr")
                for which, wdram in (("k", wkT), ("v", wvT), ("r", wrT)):
                    pss = [psum.tile([P, NCHUNK], F32, tag="ps", name=f"ps_{j}_{which}_{n}") for n in range(NT)]
                    for ci in range(NC_TILES):
                        wt = wpool.tile([P, P], MM_BITCAST, tag="w")
                        nc.sync.dma_start(wt[:], wdram[_slc(ci, P), _slc(j, P)])
                        for n in range(NT):
                            nc.tensor.matmul(
                                pss[n][:],
                                wt[:],
                                xsb[ci][:, _slc(n, NCHUNK)],
                                start=(ci == 0), stop=(ci == NC_TILES - 1),
                            )
                    for n in range(NT):
                        sl = _slc(n, NCHUNK)
                        if which == "k":
                            nc.scalar.activation(ek[:, sl], pss[n][:], AF.Exp)
                        elif which == "r":
                            nc.scalar.activation(sr[:, sl], pss[n][:], AF.Sigmoid)
                        else:  # v: ekv = exp(k) * v straight out of PSUM
                            nc.vector.tensor_tensor(
                                ekv[:, sl], ek[:, sl], pss[n][:], OP.mult)

                # ---- bidirectional scans along t ----
                ewb = ew_sb[:, j:j + 1].broadcast_to((P, T))
                eu_col = eu_sb[:, j:j + 1]
                sL = scanp.tile([P, T + 1], F32, tag="sL")
                szL = scanp.tile([P, T + 1], F32, tag="szL")
                sR = scanp.tile([P, T + 1], F32, tag="sR")
                szR = scanp.tile([P, T + 1], F32, tag="szR")
                nc.gpsimd.memset(sL[:, 0:1], 0.0)
                nc.gpsimd.memset(szL[:, 0:1], 0.0)
                nc.gpsimd.memset(sR[:, T:T + 1], 0.0)
                nc.gpsimd.memset(szR[:, T:T + 1], 0.0)
                # scans split into 2 chained halves so the first half starts
                # as soon as half of ek/ekv is evicted from PSUM
                H = T // 2
                ewh = ew_sb[:, j:j + 1].broadcast_to((P, H))
                nc.vector.tensor_tensor_scan(
                    sL[:, 1:H + 1], ewh, ekv[:, 0:H], 0.0, OP.mult, OP.add)
                nc.vector.tensor_tensor_scan(
                    szL[:, 1:H + 1], ewh, ek[:, 0:H], 0.0, OP.mult, OP.add)
                nc.vector.tensor_tensor_scan(
                    sR[:, H:T][:, ::-1], ewh, ekv[:, H:T][:, ::-1],
                    0.0, OP.mult, OP.add)
                nc.vector.tensor_tensor_scan(
                    szR[:, H:T][:, ::-1], ewh, ek[:, H:T][:, ::-1],
                    0.0, OP.mult, OP.add)
                nc.vector.tensor_tensor_scan(
                    sL[:, H + 1:T + 1], ewh, ekv[:, H:T], sL[:, H:H + 1],
                    OP.mult, OP.add)
                nc.vector.tensor_tensor_scan(
                    szL[:, H + 1:T + 1], ewh, ek[:, H:T], szL[:, H:H + 1],
                    OP.mult, OP.add)
                nc.vector.tensor_tensor_scan(
                    sR[:, 0:H][:, ::-1], ewh, ekv[:, 0:H][:, ::-1],
                    sR[:, H:H + 1], OP.mult, OP.add)
                nc.vector.tensor_tensor_scan(
                    szR[:, 0:H][:, ::-1], ewh, ek[:, 0:H][:, ::-1],
                    szR[:, H:H + 1], OP.mult, OP.add)

                # ---- combine in place: num->sL, den->szL, rec->sR ----
                nc.vector.scalar_tensor_tensor(
                    sL[:, 0:T], ekv[:], eu_col, sL[:, 0:T], OP.mult, OP.add)
                nc.vector.tensor_add(sL[:, 0:T], sL[:, 0:T], sR[:, 1:T + 1])
                nc.vector.scalar_tensor_tensor(
                    szL[:, 0:T], ek[:], eu_col, szL[:, 0:T], OP.mult, OP.add)
                nc.gpsimd.tensor_add(szL[:, 0:T], szL[:, 0:T], szR[:, 1:T + 1])
                nc.vector.reciprocal_approx_fast(sR[:, 0:T], szL[:, 0:T])
                nc.vector.tensor_mul(sL[:, 0:T], sL[:, 0:T], sR[:, 0:T])
                zt = zpool.tile([P, T], MM_BITCAST, tag=f"z{j}")
                nc.gpsimd.tensor_mul(zt[:], sL[:, 0:T], sr[:])
                zts.append(zt)

            # ---- output projection: outT[c, t] = sum_j WoT[j, c]^T z[j, t] ----
            for co in range(NC_TILES):
                pso = [psum.tile([P, NCHUNK], F32, tag="ps", name=f"pso_{co}_{n}") for n in range(NT)]
                for ji in range(NC_TILES):
                    wt = wpool.tile([P, P], MM_BITCAST, tag="w")
                    nc.sync.dma_start(wt[:], woT[_slc(ji, P), _slc(co, P)])
                    for n in range(NT):
                        nc.tensor.matmul(
                            pso[n][:],
                            wt[:],
                            zts[ji][:, _slc(n, NCHUNK)],
                            start=(ji == 0), stop=(ji == NC_TILES - 1),
                        )
                for n in range(NT):
                    ob = opool.tile([P, NCHUNK], F32, tag="ob")
                    nc.scalar.copy(ob[:], pso[n][:])
                    nc.sync.dma_start(outT[_slc(co, P), _slc(n, NCHUNK)], ob[:])

    nc.compile()
    return nc


def make_in_maps(x, Wk, Wv, Wr, Wo, decay, first):
    x = np.asarray(x, np.float32)
    wkT = np.ascontiguousarray(np.asarray(Wk, np.float32).T)
    wvT = np.ascontiguousarray(np.asarray(Wv, np.float32).T)
    wrT = np.ascontiguousarray(np.asarray(Wr, np.float32).T)
    woT = np.ascontiguousarray(np.asarray(Wo, np.float32).T)
    w64 = np.asarray(decay, np.float64) / T
    u64 = np.asarray(first, np.float64) / T
    ew = np.exp(-w64).astype(np.float32).reshape(C, 1)
    eu = np.exp(u64).astype(np.float32).reshape(C, 1)
    in_maps = []
    for b in range(N_CORES):
        in_maps.append(dict(
            xT=np.ascontiguousarray(x[b].T),
            WkT=wkT, WvT=wvT, WrT=wrT, WoT=woT, ew=ew, eu=eu,
        ))
    return in_maps


_NC_CACHE = None


def get_nc():
    global _NC_CACHE
    if _NC_CACHE is None:
        _NC_CACHE = build_kernel()
    return _NC_CACHE


def kernel(x, Wk, Wv, Wr, Wo, decay, first):
    nc = get_nc()
    in_maps = make_in_maps(x, Wk, Wv, Wr, Wo, decay, first)
    res = run_bass_kernel_spmd(nc, in_maps, list(range(N_CORES)))
    out = np.stack([res.results[b]["outT"].T for b in range(N_CORES)], axis=0)
    return np.ascontiguousarray(out)

